# revision 15
# baseline (speedup 1.0000x reference)
"""BrickTube kernel for 8x Trainium2 NeuronCores — low-rank two-stage version.

The 80-gate circuit collapses to out = x @ W (W 1024x1024, host-built in
float64 from `cores`). Key structural fact: W is a product of 80 random 4x4
gates, so its singular spectrum decays exponentially — rank 128 captures W to
2.7e-9 relative Frobenius error. The device therefore computes the two-stage
factorization

    h = x @ A   (1024 -> 128,  A = U_128 * sigma_128)
    y = h @ B   (128 -> 1024,  B = V_128^T)

which is 4x fewer MACs than the dense matmul the previous version ran.

Numerics / traffic (per core, 4096-row batch shard):
  - x is shipped in three tiers split by W-row-norm (same ordering insight as
    the dense version: small-norm rows carry little output energy):
      512 smallest rows  -> e4m3, consumed by fp8 DoubleRow matmuls (2 pairs)
      next 256 rows      -> e3m4 (4 mantissa bits, 2x more accurate than
                            e4m3; runs at fp16 speed which stage 1 can afford)
      top 256 rows       -> fp16
  - h stays on-chip: PSUM -> bf16 SBUF (per-column scales s_i folded into A's
    columns on the host; 1/s_i folded into B's rows).
  - stage 2 runs in bf16 (wide exponent range kills the underflow that a
    fp16 B would hit: B rows span sigma's 1e9 dynamic range).
  - y leaves the chip as int8: per-output-column scales c_j = 126/(4.5*||W_j||)
    are folded into B's columns, the DVE/ACT PSUM drain casts fp32->int8 with
    round-to-nearest + saturation (verified on HW), and the host divides by
    c_j. Statistical 4.5-sigma clip: y cols are exactly Gaussian, ~25 of 33M
    elements saturate. Host-simulated end-to-end rel err: 1.45e-2 (gate 2e-2).

  Traffic: x 5.24MB + y 4.19MB + weights 0.5MB ~= 9.9MB -> ~27.7us at the
  358 GB/s HBM-per-core limit. PE: 8 chunks x (2 DR + 4 fp16-rate stage-1 MMs
  + 8 bf16 stage-2 MMs) ~= 25.5us. Both ~3x below the dense version.

Device schedule (per core): software-pipelined chunks of 512 batch rows —
PE order s1(0), s1(1), s2(0), s1(2), s2(1), ..., with h drained to bf16 on
DVE in the gap that s1(j+1) covers, stage-2 PSUM drains alternating DVE/ACT,
and int8 outputs pair-DMA'd on the Sync ring while inputs stream on Scalar.
"""

import math

import ml_dtypes
import numpy as np

# ---- problem constants (hardcoded per contract) ----
B = 32768
D = 1024
N_CORES = 8
NPC = B // N_CORES  # 4096 batch rows per core

BOND = 2
Q = 10
N_LAYERS = 8
PAIRS1 = [(i, i + 1) for i in range(0, Q, 2)]
PAIRS2 = [(i, (i + 1) % Q) for i in range(1, Q, 2)]
HALF = Q // 2

R = 128  # factorization rank
N_DR = 512  # x rows in e4m3 (DoubleRow), smallest W-row-norms
N_E3 = 256  # x rows in e3m4
N_16 = D - N_DR - N_E3  # x rows in fp16
T8 = N_DR // 128  # 4 -> 2 DoubleRow pairs
T3 = N_E3 // 128  # 2
T16 = N_16 // 128  # 2
JC = NPC // 512  # 8 batch column chunks
MC = D // 128  # 8 output-row chunks
YCLIP = 4.5  # sigma clip for int8 y quantization


def build_w(cores: np.ndarray) -> np.ndarray:
    """Collapse the 80-gate circuit into W [1024, 1024] (float64),
    with out_row = x_row @ W."""
    c = cores.astype(np.float64)
    s = np.eye(D, dtype=np.float64).reshape((D,) + (BOND,) * Q)
    for layer in range(N_LAYERS):
        base = layer * Q
        for g, (i, j) in enumerate(PAIRS1):
            s = np.tensordot(s, c[base + g], axes=((i + 1, j + 1), (0, 1)))
            s = np.moveaxis(s, (-2, -1), (i + 1, j + 1))
        for g, (i, j) in enumerate(PAIRS2):
            s = np.tensordot(s, c[base + HALF + g], axes=((i + 1, j + 1), (0, 1)))
            s = np.moveaxis(s, (-2, -1), (i + 1, j + 1))
    return s.reshape(D, D)


_NC_CACHE = None


def _build_bass():
    """Device program (identical on all 8 cores). DRAM layouts are fused
    byte blocks (p = partition) so each transfer is ONE dma_start — the
    engine-side descriptor generation (~5ns x 128 partition segments) was
    the v2 bottleneck:
      xalld [128, JC*5120] bytes: per chunk [x8 e4m3 2048B | x3 e3m4 1024B
            | x16 fp16 2048B], each tier t-major: tier[p, t*512+n] =
            xq[t*128+p, j*512+n]
      walld [128, 3328] bytes: [a8 512B | a3 256B | a16 512B | b2 2048B],
            a?[p, t*R+m] = Aq[t*128+p, m]; b2[p, m] = B2q[p, m]
      ytd   [128, JC*MC*512] int8: ytd[p, (j*MC+m)*512+n] = q[j*512+n, m*128+p]
    """
    global _NC_CACHE
    if _NC_CACHE is not None:
        return _NC_CACHE

    import concourse.bacc as bacc
    import concourse.mybir as mybir
    import concourse.tile as tile

    F8 = mybir.dt.float8e4
    E3 = mybir.dt.float8e3
    F16 = mybir.dt.float16
    BF16 = mybir.dt.bfloat16
    F32 = mybir.dt.float32
    I8 = mybir.dt.int8
    DR = mybir.MatmulPerfMode.DoubleRow

    # byte offsets of the x tiers inside one fused 5120B/partition chunk
    XB8, XB3, XB16 = T8 * 512, T3 * 512, T16 * 512 * 2
    XB = XB8 + XB3 + XB16  # 5120
    # byte offsets inside the fused 3328B/partition weight block
    WB8, WB3, WB16, WBB = T8 * R, T3 * R, T16 * R * 2, D * 2
    WB = WB8 + WB3 + WB16 + WBB  # 3328

    nc = bacc.Bacc("TRN2")
    xalld = nc.dram_tensor("xalld", [128, JC * XB], I8, kind="ExternalInput")
    walld = nc.dram_tensor("walld", [128, WB], I8, kind="ExternalInput")
    ytd = nc.dram_tensor("ytd", [128, JC * MC * 512], I8, kind="ExternalOutput")

    with tile.TileContext(nc) as tc:
        with (
            tc.tile_pool(name="xpool", bufs=1) as xpool,
            tc.tile_pool(name="wpool", bufs=1) as wpool,
            tc.tile_pool(name="hpool", bufs=1) as hpool,
            tc.tile_pool(name="opool", bufs=1) as opool,
            tc.tile_pool(name="psum", bufs=1, space="PSUM") as ppool,
        ):
            # ---- PE warmup: matmuls on zeros cover the HAM clock ramp
            # while the weight + first x chunk DMAs are in flight.
            warm = xpool.tile([128, 512], F16, name="warm", tag="warm")
            nc.vector.memset(warm[:], 0)
            wps = ppool.tile([128, 2 * 512], F32, name="wps", tag="psyA")
            for _ in range(5):
                nc.tensor.matmul(wps[0:128, :512], warm[:, :128], warm[:])

            # ---- fused weight block on the Scalar ring: stage-1 weights
            # first (gate the first real MMs), b2 separately (needed later)
            wall = wpool.tile([128, WB], I8, name="wall", tag="wall")
            nc.scalar.dma_start(wall[:, : WB8 + WB3 + WB16], walld[:, : WB8 + WB3 + WB16])
            nc.scalar.dma_start(wall[:, WB8 + WB3 + WB16 :], walld[:, WB8 + WB3 + WB16 :])
            a8t = wall.bitcast(F8)[:, :WB8]
            a3t = wall.bitcast(E3)[:, WB8 : WB8 + WB3]
            a16t = wall.bitcast(F16)[:, (WB8 + WB3) // 2 : (WB8 + WB3 + WB16) // 2]
            b2t = wall.bitcast(BF16)[:, (WB8 + WB3 + WB16) // 2 : WB // 2]

            # ---- x chunks: one fused DMA per chunk, split across both
            # HWDGE rings (even j on Sync, odd j on Scalar) so early chunks
            # land in parallel; j=0 in 4 pieces for the first DR pair.
            xall = []
            for j in range(JC):
                t = xpool.tile([128, XB], I8, name=f"xall{j}", tag=f"xall{j}")
                src = xalld[:, j * XB : (j + 1) * XB]
                if j == 0:
                    nc.sync.dma_start(t[:, :1024], src[:, :1024])
                    nc.sync.dma_start(t[:, 1024:2048], src[:, 1024:2048])
                    nc.sync.dma_start(t[:, 2048:3072], src[:, 2048:3072])
                    nc.sync.dma_start(t[:, 3072:], src[:, 3072:])
                else:
                    eng = nc.sync if j % 2 == 0 else nc.scalar
                    eng.dma_start(t[:], src)
                xall.append(t)

            a8v = a8t.rearrange("p (t m) -> p t m", m=R)
            drain_ct = [0]
            pair_ct = [0]

            def drain(dst, src):
                """PSUM->SBUF drains round-robin between DVE and ACT."""
                if drain_ct[0] % 2 == 0:
                    nc.vector.tensor_scalar_mul(dst, src, 1.0)
                else:
                    nc.scalar.mul(dst, src, 1.0)
                drain_ct[0] += 1

            def s1(j):
                """Stage 1: psh[j%2] = x_chunk_j @ A (fp8 DR + e3m4 + fp16),
                then drain to bf16 h[j%2]."""
                psh = ppool.tile([128, 512], F32, name=f"psh{j%2}", tag=f"psh{j%2}")
                x8v = (
                    xall[j]
                    .bitcast(F8)[:, :XB8]
                    .rearrange("p (t n) -> p t n", n=512)
                )
                x3v = xall[j].bitcast(E3)[:, XB8 : XB8 + XB3]
                x16v = xall[j].bitcast(F16)[:, (XB8 + XB3) // 2 : XB // 2]
                for tp in range(T8 // 2):
                    nc.tensor.matmul(
                        psh[:],
                        a8v[:, 2 * tp : 2 * tp + 2, :],
                        x8v[:, 2 * tp : 2 * tp + 2, :],
                        start=(tp == 0),
                        stop=False,
                        perf_mode=DR,
                    )
                for t in range(T3):
                    nc.tensor.matmul(
                        psh[:],
                        a3t[:, t * R : (t + 1) * R],
                        x3v[:, t * 512 : (t + 1) * 512],
                        start=False,
                        stop=False,
                    )
                for t in range(T16):
                    nc.tensor.matmul(
                        psh[:],
                        a16t[:, t * R : (t + 1) * R],
                        x16v[:, t * 512 : (t + 1) * 512],
                        start=False,
                        stop=(t == T16 - 1),
                    )
                # h-drain split across BOTH engines: keeps per-iteration
                # engine loads balanced and un-queues the s2-gating drain
                h = hpool.tile([128, 512], BF16, name=f"h{j%2}", tag=f"h{j%2}")
                nc.vector.tensor_scalar_mul(h[:, :256], psh[:, :256], 1.0)
                nc.scalar.mul(h[:, 256:], psh[:, 256:], 1.0)
                return h

            def s2(j, h):
                """Stage 2: 8 bf16 MMs y_m = B2_m^T @ h into 2-bank PSUM
                pairs, one int8 drain per pair (halves drain-op overhead),
                whole-chunk output DMA on GpSimd — except the last chunk,
                whose pairs leave individually on the low-latency Sync ring
                as soon as their drain lands."""
                last = j == JC - 1
                osb = opool.tile(
                    [128, MC * 512], I8, name=f"osb{j%2}", tag=f"osb{j%2}"
                )
                names = ["psyA", "psyB", "psyC"]
                for mp in range(MC // 2):
                    # continuous global rotation: uniform 3-pair reuse
                    # distance, no adjacent reuse at chunk boundaries
                    tagp = names[pair_ct[0] % 3]
                    pair_ct[0] += 1
                    psy = ppool.tile(
                        [128, 2 * 512], F32, name=tagp, tag=tagp
                    )
                    for hh in range(2):
                        nc.tensor.matmul(
                            psy[:, hh * 512 : (hh + 1) * 512],
                            b2t[:, (2 * mp + hh) * 128 : (2 * mp + hh + 1) * 128],
                            h[:],
                            start=True,
                            stop=True,
                        )
                    dst = osb[:, mp * 1024 : (mp + 1) * 1024]
                    if last:
                        # tail: split each drain across both engines, ship
                        # each pair on the low-latency Sync ring immediately
                        nc.vector.tensor_scalar_mul(dst[:, :512], psy[:, :512], 1.0)
                        nc.scalar.mul(dst[:, 512:], psy[:, 512:], 1.0)
                        off = j * MC * 512 + mp * 1024
                        nc.sync.dma_start(ytd[:, off : off + 1024], dst)
                    else:
                        drain(dst, psy[:])
                if not last:
                    nc.gpsimd.dma_start(
                        ytd[:, j * MC * 512 : (j + 1) * MC * 512], osb[:]
                    )

            h_prev = s1(0)
            for j in range(1, JC + 1):
                h_cur = s1(j) if j < JC else None
                s2(j - 1, h_prev)
                h_prev = h_cur

    nc.compile()
    _NC_CACHE = nc
    return nc


def _prepare(x: np.ndarray, cores: np.ndarray):
    """Host-side: build W, factorize, pick the precision tiers, fold all
    quantization scales into A/B, and pack operands into the
    per-partition-contiguous device layouts."""
    W = build_w(cores)
    U, s, Vt = np.linalg.svd(W)
    A = U[:, :R] * s[:R]  # [D, R]
    Bm = Vt[:R]  # [R, D]

    rn2 = (W * W).sum(axis=1)
    order = np.argsort(rn2, kind="stable")
    sel_dr = order[:N_DR]
    sel_e3 = order[N_DR : N_DR + N_E3]
    sel_16 = order[N_DR + N_E3 :]

    A_dr, A_e3, A_16 = A[sel_dr], A[sel_e3], A[sel_16]
    xf = x.astype(np.float32)
    cx3 = 14.0 / max(float(np.abs(xf[:, sel_e3]).max()), 1e-30)

    # shared per-h-column scale s_i: min over the three format constraints
    si = 216.0 / np.maximum(np.abs(A_dr).max(axis=0), 1e-30)
    si = np.minimum(si, 15.0 * cx3 / np.maximum(np.abs(A_e3).max(axis=0), 1e-30))
    si = np.minimum(si, 30000.0 / np.maximum(np.abs(A_16).max(axis=0), 1e-30))

    E4NP = ml_dtypes.float8_e4m3
    E3NP = ml_dtypes.float8_e3m4

    def to_dram_w(Aq, tcount, np_dt):
        # a?d[p, t*R + m] = Aq[t*128+p, m]
        return np.ascontiguousarray(
            Aq.astype(np.float32)
            .astype(np_dt)
            .reshape(tcount, 128, R)
            .transpose(1, 0, 2)
            .reshape(128, tcount * R)
        )

    a8d = to_dram_w(np.clip(A_dr * si, -240, 240), T8, E4NP)
    a3d = to_dram_w(np.clip(A_e3 * (si / cx3), -15.5, 15.5), T3, E3NP)
    a16d = to_dram_w(A_16 * si, T16, np.float16)

    # stage-2 weights: fold 1/s_i (rows) and y-column scales c_j (columns)
    wcol = np.sqrt((W * W).sum(axis=0))
    cj = 126.0 / (YCLIP * np.maximum(wcol, 1e-30))
    b2d = np.ascontiguousarray(
        (Bm * cj[None, :] / si[:, None]).astype(np.float32).astype(ml_dtypes.bfloat16)
    )

    # fused weight block: [a8 | a3 | a16 | b2] as raw bytes per partition
    walld = np.ascontiguousarray(
        np.concatenate(
            [
                a8d.view(np.uint8),
                a3d.view(np.uint8),
                a16d.view(np.uint8),
                b2d.view(np.uint8),
            ],
            axis=1,
        )
    )

    # x tiers, transposed to [rows, B]
    x8_full = xf[:, sel_dr].astype(E4NP).T
    x3_full = (xf[:, sel_e3] * cx3).astype(E3NP).T
    x16_full = xf[:, sel_16].astype(np.float16).T
    return walld, x8_full, x3_full, x16_full, cj


def _pack_x(xf: np.ndarray, c: int, tcount: int):
    """[tcount*128, B] core shard -> [128, JC, tcount*512*itemsize] bytes."""
    shard = xf[:, c * NPC : (c + 1) * NPC]
    packed = np.ascontiguousarray(
        shard.reshape(tcount, 128, JC, 512).transpose(1, 2, 0, 3)
    )
    return packed.view(np.uint8).reshape(128, JC, -1)


def _run(x: np.ndarray, cores: np.ndarray, trace: bool = False, trace_cores=None):
    from concourse.bass_utils import run_bass_kernel_spmd

    walld, x8_full, x3_full, x16_full, cj = _prepare(x, cores)

    in_maps = []
    for c in range(N_CORES):
        xall = np.concatenate(
            [
                _pack_x(x8_full, c, T8),
                _pack_x(x3_full, c, T3),
                _pack_x(x16_full, c, T16),
            ],
            axis=2,
        ).reshape(128, -1)
        in_maps.append(
            {"xalld": xall.view(np.int8), "walld": walld.view(np.int8)}
        )

    nc = _build_bass()
    kwargs = {}
    if trace_cores is not None:
        kwargs["trace_cores"] = trace_cores
    res = run_bass_kernel_spmd(
        nc, in_maps, core_ids=list(range(N_CORES)), trace=trace, **kwargs
    )

    inv_cj = (1.0 / cj).astype(np.float32)
    y = np.empty((B, D), dtype=np.float32)
    for c in range(N_CORES):
        # ytd[p, (j*MC+m)*512+n] = q[j*512+n, m*128+p]
        arr = res.results[c]["ytd"].reshape(128, JC, MC, 512)
        q = arr.transpose(1, 3, 2, 0).reshape(NPC, D)
        y[c * NPC : (c + 1) * NPC, :] = q.astype(np.float32) * inv_cj[None, :]
    return y, res


def kernel(x: np.ndarray, cores: np.ndarray) -> np.ndarray:
    y, _ = _run(x, cores, trace=False)
    return y
